# revision 36
# baseline (speedup 1.0000x reference)
"""MinGRU forward on 8 Trainium2 NeuronCores.

Reference computation (per batch b):
    k       = x @ Wz + bz                 # [T, H]
    z       = sigmoid(k)
    c       = 1 - z
    htilde  = g(x @ Wh + bh)              # g(a) = a+0.5 if a>=0 else sigmoid(a)
                                          #      = max(a+0.5, sigmoid(a))
    h[0]    = g(h_0)
    h[t]    = c[t-1]*h[t-1] + z[t-1]*htilde[t-1]   (t = 1..T)
    out     = h                           # [T+1, H]

The log-space cumlogsumexp in the reference is exactly this linear
recurrence (all quantities positive, coefficients in (0,1), so the
linear form is numerically stable).

Sharding: data-parallel over batch, one batch per core, weights
replicated.

The kernel is Tensor-engine bound: 1024 fp16 matmuls/core = 218.5us at
2.4GHz. fp8 DoubleRow was measured on hardware at ~1 cycle/output-row
(2x FLOPs via 256-deep contraction, not the cost model's 4x), so
error-compensated hi/lo fp8 (3 logical matmuls, verified numerically
at 0.008 max rel err) is 1.5x SLOWER than fp16 — fp16 is optimal.
The optimization is therefore all PE-occupancy at the edges
(257.7us -> 245.1us measured):
  - x is transposed AND cast to fp16 on the host, so the device issues
    only plain contiguous DMAs. The baseline's device-side DMA-transpose
    serialized the weight loads behind it, costing ~8us of PE idle at
    kernel start.
  - DMA priority: weights stream ALONE on the in-order sync ring,
    ordered exactly as chunk 0's k-outer schedule consumes them (all
    low m-halves first, wh0's low half split so the first matmul waits
    on 32KB). ALL x traffic (chunk 0 slice-by-slice, then whole
    chunks) rides the ACT ring — interleaving chunk-0 x into the
    weight ring starved the weight stream mid-chunk-0 (~1us of PE
    gaps at the lo->hi half switch); x slice 0 is split in halves and
    ko=0's matmuls t-split so the PE starts on the first 64KB.
    Small constants are host-packed into one [128, 24] block — the
    natural per-vector rearranges emit 1024 4-byte scatter descriptors
    that starve the critical first weight slice.
  - Gates run fp16 end to end (z, s, c, g, v, h): DVE gets 2x
    throughput on 16-bit SBUF operands, the output DMA halves, and ACT
    drops from 3 sigmoids to 2 (c = 1-z moves to a cheap DVE
    tensor_scalar). GpSimd compute is not used at all (its software
    multiply is ~4x slower than DVE fp16).
  - The scan keeps fp32 state internally (hardware guarantee) and only
    stores h as fp16; rel err stays ~4.6e-3 (limit 2e-2).
  - The last 512 timesteps run as two 256 chunks and the final tile's
    gates in two 128 slices, so the post-matmul tail chain is short.
  - HAM pre-warm: 18 ZERO-data matmuls bridge the preamble-to-first-
    data window so the clock gate flips to 2.4GHz right at the real
    stream's start. Zeroed operands = no array switching, so this does
    NOT trip the power envelope the way a warmup on real data did
    (+38us, 13/16-duty); measured clean 4/8 -> 8/8 transitions.
  - Chunk 0 is marginally DMA-bound (~5MB of weights+x must stream
    during its ~27.6us of matmuls at ~200GB/s early ring speed), so
    chunk 0 runs ALL wh matmuls of a half before any wz matmuls and
    the weight stream is ordered to match: the early window then only
    has to keep up with one weight matrix while the pipeline fills.
The device writes timesteps 1..T transposed ([H, T] fp16); the host
prepends g(h_0), transposes and upcasts during the unshard.
"""

import numpy as np

B, T, D, H = 8, 4096, 1024, 1024
P = 128
TCH = 512                 # time-chunk (one PSUM bank of fp32 per matmul)
KO = D // P               # contraction tiles
MO = H // P               # output-channel tiles
# 7 full chunks + 2 half chunks at the end to shorten the tail
CHUNKS = [(i * TCH, TCH) for i in range(7)] + [(3584, 256), (3840, 256)]
NTCH = T // TCH           # host x layout is uniform 512-chunk-major

_PROGRAM_CACHE = {}


def _build_program():
    import concourse.bacc as bacc
    import concourse.mybir as mybir
    import concourse.tile as tile

    fp32 = mybir.dt.float32
    fp16 = mybir.dt.float16
    SIG = mybir.ActivationFunctionType.Sigmoid
    MUL = mybir.AluOpType.mult
    ADD = mybir.AluOpType.add
    MAX = mybir.AluOpType.max

    nc = bacc.Bacc("TRN2", target_bir_lowering=False)

    # x pre-transposed on host: [ki, nt, ko, t] with D-index = ko*128+ki,
    # T-index = nt*512+t  (chunk-major so each chunk DMA reads 8KB runs)
    xt_ext = nc.declare_dram_parameter("xt", [P, NTCH * KO * TCH], fp16, isOutput=False)
    wz_ext = nc.declare_dram_parameter("Wz", [D, H], fp16, isOutput=False)
    wh_ext = nc.declare_dram_parameter("Wh", [D, H], fp16, isOutput=False)
    # host-packed [bz_t | bh_t | h0_t] in device layout (partition = channel
    # within tile, free = tile): a single small contiguous DMA. The natural
    # per-tensor rearranges generate 1024 4-byte scatter descriptors each,
    # which hogged the DMA engines right when the first weight slice's bulk
    # data needed them.
    cst_ext = nc.declare_dram_parameter("cst", [P, 3 * MO], fp32, isOutput=False)
    # transposed fp16 output, timesteps 1..T; the host prepends g(h_0) and
    # untransposes/upcasts during the gather
    out_ext = nc.declare_dram_parameter("out", [H, T], fp16, isOutput=True)

    xt_r = xt_ext.rearrange("p (nt ko t) -> p nt ko t", nt=NTCH, ko=KO)

    with tile.TileContext(nc) as tc:
        with (
            tc.tile_pool(name="const", bufs=1) as const_pool,
            tc.tile_pool(name="w", bufs=1) as w_pool,
            tc.tile_pool(name="xt", bufs=3) as xt_pool,
            tc.tile_pool(name="ht", bufs=2) as ht_pool,
            tc.tile_pool(name="gate", bufs=6) as gate_pool,
            tc.tile_pool(name="psp", bufs=4, space="PSUM") as psum_p,
        ):
            # HAM pre-warm: ~13 zero-data matmuls run back-to-back from
            # right after the preamble until the first real matmul's data
            # lands (~3.5us in). The PE activity window then flips the
            # clock gate to 8/8 (2.4GHz) ~4-6us into the window instead
            # of ~10us, shaving 1-2.5us of half-rate matmuls. Zeroed
            # operands keep array switching (= power draw) near nil, so
            # this does not trip the power-envelope throttle the way a
            # warmup on real data during the DMA burst did (+38us).
            warm_sb = const_pool.tile([P, 384], fp16)
            nc.gpsimd.memset(warm_sb, 0.0)
            warm_ps = psum_p.tile([P, TCH], fp32, tag="pk", name="pk")
            for _ in range(18):
                nc.tensor.matmul(
                    warm_ps[:, 0:256], warm_sb[:, 0:P], warm_sb[:, P:P + 256],
                    start=True, stop=True,
                )

            # Chunk 0's x rides the ACT ring, in two halves so the very
            # first matmuls (split into t-halves for ko=0) wait on only
            # 64KB. The sync ring carries ONLY weights in its early
            # window: interleaving chunk-0 x there starved the weight
            # stream mid-chunk-0 (~1us of PE gaps at the lo->hi switch).
            xt_first = xt_pool.tile([P, KO, TCH], fp16, tag="xt512", name="xt512")
            TH = TCH // 2
            nc.scalar.dma_start(xt_first[:, 0, 0:TH], xt_r[:, 0, 0, 0:TH])
            nc.scalar.dma_start(xt_first[:, 0, TH:], xt_r[:, 0, 0, TH:])

            # Weights resident: [ki, ko, h] so lhsT tiles are natural slices.
            # Loaded per k-slice (contiguous 256KB each) on the sync ring, in
            # the order chunk 0's k-outer matmul schedule consumes them
            # (pa/wh first). The sync ring carries nothing else early, so
            # the ~130GB/s the stream needs is comfortably under the ring's
            # ~200GB/s and the PE never waits on a weight slice.
            wz_sb = w_pool.tile([P, KO, H], fp16)
            wh_sb = w_pool.tile([P, KO, H], fp16)
            wz_r = wz_ext.rearrange("(ko ki) h -> ki ko h", ki=P)
            wh_r = wh_ext.rearrange("(ko ki) h -> ki ko h", ki=P)
            # Stream order matches chunk 0's k-outer consumption exactly:
            # the first half (m-tiles 0-3) of every k-slice first — wh0's
            # low half further split so the very first matmuls wait on
            # 32KB/96KB — then all high halves. Half 0 of chunk 0 then
            # needs only 2MB of weights in its window instead of 4MB.
            # Chunk 0 runs ALL wh (pa) matmuls of a half before any wz
            # (pk) matmuls, so the early stream only has to keep up with
            # ONE weight matrix (~145GB/s) while the pipeline fills —
            # with a warm (pre-warmed) PE the interleaved order outran
            # the stream and left ~2us of gaps. Stream order matches.
            # Hybrid stream: the spin-up-critical slices (wh0-wh3 lo, all
            # the first matmuls can touch in the ring's slow first ~7us)
            # stay fine-grained so their completion sems fire early; the
            # remaining ~3.4MB rides FULL-H k-slices whose 2KB contiguous
            # runs stream at ~380GB/s (1KB runs only manage ~200GB/s —
            # the ring is descriptor-run-size bound), so the whole weight
            # set lands ~14us before the old lo/hi-half stream finished
            # and the warm PE can no longer outrun it mid-chunk-0.
            HH = H // 2
            nc.sync.dma_start(wh_sb[:, 0, 0:P], wh_r[:, 0, 0:P])
            nc.sync.dma_start(wh_sb[:, 0, P:HH], wh_r[:, 0, P:HH])
            for ko in range(1, 4):
                nc.sync.dma_start(wh_sb[:, ko, 0:HH], wh_r[:, ko, 0:HH])
            for ko in range(4, KO):
                nc.sync.dma_start(wh_sb[:, ko, :], wh_r[:, ko, :])
            for ko in range(KO):
                nc.sync.dma_start(wz_sb[:, ko, :], wz_r[:, ko, :])
            for ko in range(4):
                nc.sync.dma_start(wh_sb[:, ko, HH:], wh_r[:, ko, HH:])

            # Chunk-0 x slices ko=1..7 follow on the ACT ring (ko=1 is
            # needed ~1us after the first matmul); the small constants
            # DMA rides behind them (first needed only at the gates,
            # ~10us later).
            for ko in range(1, KO):
                nc.scalar.dma_start(xt_first[:, ko], xt_r[:, 0, ko])
            cst_sb = const_pool.tile([P, 3 * MO], fp32)
            nc.scalar.dma_start(cst_sb, cst_ext[:, :])
            bz_sb = cst_sb[:, 0:MO]
            bh_sb = cst_sb[:, MO:2 * MO]
            h0_sb = cst_sb[:, 2 * MO:3 * MO]
            bhp5_sb = const_pool.tile([P, MO], fp32)
            nc.vector.tensor_scalar_add(bhp5_sb, bh_sb, 0.5)

            # g(h_0) for the chunk-0 scan init (out column 0 is host-side)
            s0_sb = const_pool.tile([P, MO], fp32)
            nc.scalar.activation(s0_sb, h0_sb, SIG)
            gh0_sb = const_pool.tile([P, MO], fp32)
            nc.vector.scalar_tensor_tensor(gh0_sb, h0_sb, 0.5, s0_sb, op0=ADD, op1=MAX)

            # x chunks prefetched two ahead on the GpSimd ring. Tiles are
            # tagged by width so the 256-wide tail chunks rotate separately.
            xt_tiles = [xt_first]

            def issue_xt(ci):
                t0, tch = CHUNKS[ci]
                xt_sb = xt_pool.tile(
                    [P, KO, tch], fp16, tag=f"xt{tch}", name=f"xt{tch}"
                )
                nt, toff = divmod(t0, TCH)
                # chunks 1+ queue BEHIND the weights on the in-order sync
                # ring: a parallel ring races the weight stream for the
                # shared ~250GB/s DMA bandwidth and starves it (measured
                # 5.5us PE gap + a HAM re-throttle mid-chunk-0).
                nc.sync.dma_start(xt_sb, xt_r[:, nt, :, toff:toff + tch])
                xt_tiles.append(xt_sb)

            issue_xt(1)

            prev_ht = None  # previous chunk's scan output (carries the state)
            prev_tch = TCH

            def gates_scan_store(m, t0, tch, pk, pa, ht_sb, split=1):
                # split>1 processes the tile in column slices so the very
                # last tile's gate/scan/DMA chain after the final matmul is
                # short. ACT order s -> z: s feeds the longest chain
                # (g -> v -> scan). For the split (= final) tile the two
                # slice DMAs go out on different DGE rings so their ~600ns
                # descriptor generations run in parallel.
                w = tch // split
                for si in range(split):
                    lo, hi = si * w, (si + 1) * w
                    s_sb = gate_pool.tile([P, TCH], fp16, tag="s", name="s")[:, :w]
                    nc.scalar.activation(s_sb, pa[:, lo:hi], SIG, bias=bh_sb[:, m:m + 1])
                    z_sb = gate_pool.tile([P, TCH], fp16, tag="z", name="z")[:, :w]
                    nc.scalar.activation(z_sb, pk[:, lo:hi], SIG, bias=bz_sb[:, m:m + 1])
                    # g = max(a + bh + 0.5, s) straight from PSUM
                    g_sb = gate_pool.tile([P, TCH], fp16, tag="g", name="g")[:, :w]
                    nc.vector.scalar_tensor_tensor(
                        g_sb, pa[:, lo:hi], bhp5_sb[:, m:m + 1], s_sb, op0=ADD, op1=MAX
                    )
                    c_sb = gate_pool.tile([P, TCH], fp16, tag="c", name="c")[:, :w]
                    nc.vector.tensor_scalar(c_sb, z_sb, -1.0, 1.0, op0=MUL, op1=ADD)
                    v_sb = gate_pool.tile([P, TCH], fp16, tag="v", name="v")[:, :w]
                    nc.vector.tensor_mul(v_sb, z_sb, g_sb)

                    init = (
                        gh0_sb[:, m:m + 1]
                        if prev_ht is None
                        else prev_ht[:, m, prev_tch - 1:prev_tch]
                    ) if si == 0 else ht_sb[:, m, lo - 1:lo]
                    nc.vector.tensor_tensor_scan(
                        ht_sb[:, m, lo:hi], c_sb, v_sb, init, op0=MUL, op1=ADD
                    )
                    # The last m-tile's outputs ride the ACT ring: it keeps
                    # that ring warm all run (a ~200us-idle ring pays ~1us
                    # of restart latency on the final, fully-exposed DMA)
                    # and the final two slice DMAs' ~600ns descriptor
                    # generations run on different engines in parallel.
                    if split > 1:
                        # final tile: alternate rings so the two DGEs overlap
                        eng = nc.scalar if si == split - 1 else nc.sync
                    else:
                        eng = nc.scalar if m == MO - 1 else nc.sync
                    eng.dma_start(
                        out_ext[m * P:(m + 1) * P, t0 + lo:t0 + hi],
                        ht_sb[:, m, lo:hi],
                    )

            for ci, (t0, tch) in enumerate(CHUNKS):
                if ci + 2 < len(CHUNKS):
                    issue_xt(ci + 2)
                xt_sb = xt_tiles[ci]
                ht_sb = ht_pool.tile([P, MO, TCH], fp16)

                if ci == 0:
                    # k-outer over groups of 4 output tiles: matmuls consume
                    # the weight k-slices in DMA arrival order, so the PE
                    # starts ~15us earlier and never stalls on weight loads
                    # (which would also re-throttle the HAM clock gate).
                    # ko=0 of the first group is further split into t-halves
                    # so the very first matmuls wait on only half of x
                    # slice 0 (the second half's DMA finishes under them).
                    GQ = min(4, MO)
                    for half in range(MO // GQ):
                        pks = [
                            psum_p.tile([P, TCH], fp32, tag="pk", name="pk")
                            for _ in range(GQ)
                        ]
                        pas = [
                            psum_p.tile([P, TCH], fp32, tag="pa", name="pa")
                            for _ in range(GQ)
                        ]
                        # ALL pa (wh) matmuls of the half first, then all
                        # pk (wz): the early window only needs one weight
                        # matrix streamed. pa first also because s/g/v
                        # feed the longest downstream chain.
                        for ps, wsb in ((pas, wh_sb), (pks, wz_sb)):
                            for ko in range(KO):
                                if ko == 0 and half == 0:
                                    # t-split: half0 with start=True clears
                                    # the whole bank's has_written bits;
                                    # half1 (start=False) then overwrites
                                    # its cleared columns, and ko>=1
                                    # accumulates everywhere.
                                    for th in range(2):
                                        sl = slice(th * TH, (th + 1) * TH)
                                        for q in range(GQ):
                                            m = half * GQ + q
                                            nc.tensor.matmul(
                                                ps[q][:, sl],
                                                wsb[:, 0, m * P:(m + 1) * P],
                                                xt_sb[:, 0, sl],
                                                start=(th == 0),
                                                stop=False,
                                            )
                                    continue
                                for q in range(GQ):
                                    m = half * GQ + q
                                    nc.tensor.matmul(
                                        ps[q],
                                        wsb[:, ko, m * P:(m + 1) * P],
                                        xt_sb[:, ko, :],
                                        start=(ko == 0),
                                        stop=(ko == KO - 1),
                                    )
                        for q in range(GQ):
                            m = half * GQ + q
                            gates_scan_store(m, t0, tch, pks[q], pas[q], ht_sb)
                else:
                    for m in range(MO):
                        pk = psum_p.tile([P, TCH], fp32, tag="pk", name="pk")[:, :tch]
                        pa = psum_p.tile([P, TCH], fp32, tag="pa", name="pa")[:, :tch]
                        for ko in range(KO):
                            nc.tensor.matmul(
                                pa,
                                wh_sb[:, ko, m * P:(m + 1) * P],
                                xt_sb[:, ko, :tch],
                                start=(ko == 0),
                                stop=(ko == KO - 1),
                            )
                        for ko in range(KO):
                            nc.tensor.matmul(
                                pk,
                                wz_sb[:, ko, m * P:(m + 1) * P],
                                xt_sb[:, ko, :tch],
                                start=(ko == 0),
                                stop=(ko == KO - 1),
                            )
                        last = ci == len(CHUNKS) - 1 and m == MO - 1
                        gates_scan_store(m, t0, tch, pk, pa, ht_sb,
                                         split=2 if last else 1)

                prev_ht = ht_sb
                prev_tch = tch

    nc.finalize()
    return nc


def _get_program():
    if "v15" not in _PROGRAM_CACHE:
        _PROGRAM_CACHE["v15"] = _build_program()
    return _PROGRAM_CACHE["v15"]


def _prep_xt(xb):
    # [T, D] fp32 -> fp16 [ki, nt, ko, tch] with D = ko*128+ki, T = nt*512+tch
    x16 = np.asarray(xb, dtype=np.float16)
    xt = x16.reshape(NTCH, TCH, KO, P).transpose(3, 0, 2, 1)
    return np.ascontiguousarray(xt).reshape(P, NTCH * KO * TCH)


def run(x, h_0, Wz, bz, Wh, bh, trace=False):
    from concourse.bass_utils import run_bass_kernel_spmd

    nc = _get_program()
    wz16 = np.ascontiguousarray(np.asarray(Wz, dtype=np.float16))
    wh16 = np.ascontiguousarray(np.asarray(Wh, dtype=np.float16))
    bz32 = np.asarray(bz, dtype=np.float32)
    bh32 = np.asarray(bh, dtype=np.float32)
    h0_32 = np.asarray(h_0, dtype=np.float32).reshape(B, H)

    def dev_cols(v):  # [H] -> [P, MO] device layout (partition = mi)
        return v.reshape(MO, P).T

    in_maps = [
        {
            "xt": _prep_xt(x[b]),
            "Wz": wz16,
            "Wh": wh16,
            "cst": np.ascontiguousarray(np.concatenate(
                [dev_cols(bz32), dev_cols(bh32), dev_cols(h0_32[b])], axis=1
            )),
        }
        for b in range(B)
    ]
    res = run_bass_kernel_spmd(nc, in_maps, list(range(B)), trace=trace)
    out = np.empty((B, T + 1, H), dtype=np.float32)
    # h[0] = g(h_0) computed on host in fp32
    out[:, 0, :] = np.where(
        h0_32 >= 0.0, h0_32 + 0.5, 1.0 / (1.0 + np.exp(-h0_32))
    )
    for b in range(B):
        out[b, 1:, :] = res.results[b]["out"].T.astype(np.float32)
    return out, res


def kernel(x, h_0, Wz, bz, Wh, bh):
    out, _ = run(x, h_0, Wz, bz, Wh, bh)
    return out



# revision 38
# speedup vs baseline: 1.0131x; 1.0131x over previous
"""MinGRU forward on 8 Trainium2 NeuronCores.

Reference computation (per batch b):
    k       = x @ Wz + bz                 # [T, H]
    z       = sigmoid(k)
    c       = 1 - z
    htilde  = g(x @ Wh + bh)              # g(a) = a+0.5 if a>=0 else sigmoid(a)
                                          #      = max(a+0.5, sigmoid(a))
    h[0]    = g(h_0)
    h[t]    = c[t-1]*h[t-1] + z[t-1]*htilde[t-1]   (t = 1..T)
    out     = h                           # [T+1, H]

The log-space cumlogsumexp in the reference is exactly this linear
recurrence (all quantities positive, coefficients in (0,1), so the
linear form is numerically stable).

Sharding: data-parallel over batch, one batch per core, weights
replicated.

The kernel is Tensor-engine bound: 1024 fp16 matmuls/core = 218.5us at
2.4GHz. fp8 DoubleRow was measured on hardware at ~1 cycle/output-row
(2x FLOPs via 256-deep contraction, not the cost model's 4x), so
error-compensated hi/lo fp8 (3 logical matmuls, verified numerically
at 0.008 max rel err) is 1.5x SLOWER than fp16 — fp16 is optimal.
The optimization is therefore all PE-occupancy at the edges
(257.7us -> 245.1us measured):
  - x is transposed AND cast to fp16 on the host, so the device issues
    only plain contiguous DMAs. The baseline's device-side DMA-transpose
    serialized the weight loads behind it, costing ~8us of PE idle at
    kernel start.
  - DMA priority: weights stream ALONE on the in-order sync ring,
    ordered exactly as chunk 0's k-outer schedule consumes them (all
    low m-halves first, wh0's low half split so the first matmul waits
    on 32KB). ALL x traffic (chunk 0 slice-by-slice, then whole
    chunks) rides the ACT ring — interleaving chunk-0 x into the
    weight ring starved the weight stream mid-chunk-0 (~1us of PE
    gaps at the lo->hi half switch); x slice 0 is split in halves and
    ko=0's matmuls t-split so the PE starts on the first 64KB.
    Small constants are host-packed into one [128, 24] block — the
    natural per-vector rearranges emit 1024 4-byte scatter descriptors
    that starve the critical first weight slice.
  - Gates run fp16 end to end (z, s, c, g, v, h): DVE gets 2x
    throughput on 16-bit SBUF operands, the output DMA halves, and ACT
    drops from 3 sigmoids to 2 (c = 1-z moves to a cheap DVE
    tensor_scalar). GpSimd compute is not used at all (its software
    multiply is ~4x slower than DVE fp16).
  - The scan keeps fp32 state internally (hardware guarantee) and only
    stores h as fp16; rel err stays ~4.6e-3 (limit 2e-2).
  - The last 512 timesteps run as two 256 chunks and the final tile's
    gates in two 128 slices, so the post-matmul tail chain is short.
  - HAM pre-warm: 18 ZERO-data matmuls bridge the preamble-to-first-
    data window so the clock gate flips to 2.4GHz right at the real
    stream's start. Zeroed operands = no array switching, so this does
    NOT trip the power envelope the way a warmup on real data did
    (+38us, 13/16-duty); measured clean 4/8 -> 8/8 transitions.
  - Chunk 0 is marginally DMA-bound (~5MB of weights+x must stream
    during its ~27.6us of matmuls at ~200GB/s early ring speed), so
    chunk 0 runs ALL wh matmuls of a half before any wz matmuls and
    the weight stream is ordered to match: the early window then only
    has to keep up with one weight matrix while the pipeline fills.
The device writes timesteps 1..T transposed ([H, T] fp16); the host
prepends g(h_0), transposes and upcasts during the unshard.
"""

import numpy as np

B, T, D, H = 8, 4096, 1024, 1024
P = 128
TCH = 512                 # time-chunk (one PSUM bank of fp32 per matmul)
KO = D // P               # contraction tiles
MO = H // P               # output-channel tiles
# 7 full chunks + 2 half chunks at the end to shorten the tail
CHUNKS = [(i * TCH, TCH) for i in range(7)] + [(3584, 256), (3840, 256)]
NTCH = T // TCH           # host x layout is uniform 512-chunk-major

_PROGRAM_CACHE = {}


def _build_program():
    import concourse.bacc as bacc
    import concourse.mybir as mybir
    import concourse.tile as tile

    fp32 = mybir.dt.float32
    fp16 = mybir.dt.float16
    SIG = mybir.ActivationFunctionType.Sigmoid
    MUL = mybir.AluOpType.mult
    ADD = mybir.AluOpType.add
    MAX = mybir.AluOpType.max

    nc = bacc.Bacc("TRN2", target_bir_lowering=False)

    # x pre-transposed on host: [ki, nt, ko, t] with D-index = ko*128+ki,
    # T-index = nt*512+t  (chunk-major so each chunk DMA reads 8KB runs)
    xt_ext = nc.declare_dram_parameter("xt", [P, NTCH * KO * TCH], fp16, isOutput=False)
    wz_ext = nc.declare_dram_parameter("Wz", [D, H], fp16, isOutput=False)
    wh_ext = nc.declare_dram_parameter("Wh", [D, H], fp16, isOutput=False)
    # host-packed [bz_t | bh_t | h0_t] in device layout (partition = channel
    # within tile, free = tile): a single small contiguous DMA. The natural
    # per-tensor rearranges generate 1024 4-byte scatter descriptors each,
    # which hogged the DMA engines right when the first weight slice's bulk
    # data needed them.
    cst_ext = nc.declare_dram_parameter("cst", [P, 3 * MO], fp32, isOutput=False)
    # transposed fp16 output, timesteps 1..T; the host prepends g(h_0) and
    # untransposes/upcasts during the gather
    out_ext = nc.declare_dram_parameter("out", [H, T], fp16, isOutput=True)

    xt_r = xt_ext.rearrange("p (nt ko t) -> p nt ko t", nt=NTCH, ko=KO)

    with tile.TileContext(nc) as tc:
        with (
            tc.tile_pool(name="const", bufs=1) as const_pool,
            tc.tile_pool(name="w", bufs=1) as w_pool,
            tc.tile_pool(name="xt", bufs=3) as xt_pool,
            tc.tile_pool(name="ht", bufs=2) as ht_pool,
            tc.tile_pool(name="gate", bufs=8) as gate_pool,
            tc.tile_pool(name="psp", bufs=4, space="PSUM") as psum_p,
        ):
            # HAM pre-warm: ~13 zero-data matmuls run back-to-back from
            # right after the preamble until the first real matmul's data
            # lands (~3.5us in). The PE activity window then flips the
            # clock gate to 8/8 (2.4GHz) ~4-6us into the window instead
            # of ~10us, shaving 1-2.5us of half-rate matmuls. Zeroed
            # operands keep array switching (= power draw) near nil, so
            # this does not trip the power-envelope throttle the way a
            # warmup on real data during the DMA burst did (+38us).
            warm_sb = const_pool.tile([P, 384], fp16)
            nc.gpsimd.memset(warm_sb, 0.0)
            warm_ps = psum_p.tile([P, TCH], fp32, tag="pk", name="pk")
            for _ in range(18):
                nc.tensor.matmul(
                    warm_ps[:, 0:256], warm_sb[:, 0:P], warm_sb[:, P:P + 256],
                    start=True, stop=True,
                )

            # Chunk 0's x rides the ACT ring, in two halves so the very
            # first matmuls (split into t-halves for ko=0) wait on only
            # 64KB. The sync ring carries ONLY weights in its early
            # window: interleaving chunk-0 x there starved the weight
            # stream mid-chunk-0 (~1us of PE gaps at the lo->hi switch).
            xt_first = xt_pool.tile([P, KO, TCH], fp16, tag="xt512", name="xt512")
            TH = TCH // 2
            nc.scalar.dma_start(xt_first[:, 0, 0:TH], xt_r[:, 0, 0, 0:TH])
            nc.scalar.dma_start(xt_first[:, 0, TH:], xt_r[:, 0, 0, TH:])

            # Weights resident: [ki, ko, h] so lhsT tiles are natural slices.
            # Loaded per k-slice (contiguous 256KB each) on the sync ring, in
            # the order chunk 0's k-outer matmul schedule consumes them
            # (pa/wh first). The sync ring carries nothing else early, so
            # the ~130GB/s the stream needs is comfortably under the ring's
            # ~200GB/s and the PE never waits on a weight slice.
            wz_sb = w_pool.tile([P, KO, H], fp16)
            wh_sb = w_pool.tile([P, KO, H], fp16)
            wz_r = wz_ext.rearrange("(ko ki) h -> ki ko h", ki=P)
            wh_r = wh_ext.rearrange("(ko ki) h -> ki ko h", ki=P)
            # Stream order matches chunk 0's k-outer consumption exactly:
            # the first half (m-tiles 0-3) of every k-slice first — wh0's
            # low half further split so the very first matmuls wait on
            # 32KB/96KB — then all high halves. Half 0 of chunk 0 then
            # needs only 2MB of weights in its window instead of 4MB.
            # Chunk 0 runs ALL wh (pa) matmuls of a half before any wz
            # (pk) matmuls, so the early stream only has to keep up with
            # ONE weight matrix (~145GB/s) while the pipeline fills —
            # with a warm (pre-warmed) PE the interleaved order outran
            # the stream and left ~2us of gaps. Stream order matches.
            HH = H // 2
            nc.sync.dma_start(wh_sb[:, 0, 0:P], wh_r[:, 0, 0:P])
            nc.sync.dma_start(wh_sb[:, 0, P:HH], wh_r[:, 0, P:HH])
            for ko in range(1, KO):
                nc.sync.dma_start(wh_sb[:, ko, 0:HH], wh_r[:, ko, 0:HH])
            for ko in range(KO):
                nc.sync.dma_start(wz_sb[:, ko, 0:HH], wz_r[:, ko, 0:HH])
            for ko in range(KO):
                nc.sync.dma_start(wh_sb[:, ko, HH:], wh_r[:, ko, HH:])
            for ko in range(KO):
                nc.sync.dma_start(wz_sb[:, ko, HH:], wz_r[:, ko, HH:])

            # Chunk-0 x slices ko=1..7 follow on the ACT ring (ko=1 is
            # needed ~1us after the first matmul); the small constants
            # DMA rides behind them (first needed only at the gates,
            # ~10us later).
            for ko in range(1, KO):
                nc.scalar.dma_start(xt_first[:, ko], xt_r[:, 0, ko])
            cst_sb = const_pool.tile([P, 3 * MO], fp32)
            nc.scalar.dma_start(cst_sb, cst_ext[:, :])
            bz_sb = cst_sb[:, 0:MO]
            bh_sb = cst_sb[:, MO:2 * MO]
            h0_sb = cst_sb[:, 2 * MO:3 * MO]
            bhp5_sb = const_pool.tile([P, MO], fp32)
            nc.vector.tensor_scalar_add(bhp5_sb, bh_sb, 0.5)

            # g(h_0) for the chunk-0 scan init (out column 0 is host-side)
            s0_sb = const_pool.tile([P, MO], fp32)
            nc.scalar.activation(s0_sb, h0_sb, SIG)
            gh0_sb = const_pool.tile([P, MO], fp32)
            nc.vector.scalar_tensor_tensor(gh0_sb, h0_sb, 0.5, s0_sb, op0=ADD, op1=MAX)

            # x chunks prefetched two ahead on the GpSimd ring. Tiles are
            # tagged by width so the 256-wide tail chunks rotate separately.
            xt_tiles = [xt_first]

            def issue_xt(ci):
                t0, tch = CHUNKS[ci]
                xt_sb = xt_pool.tile(
                    [P, KO, tch], fp16, tag=f"xt{tch}", name=f"xt{tch}"
                )
                nt, toff = divmod(t0, TCH)
                # chunks 1+ queue BEHIND the weights on the in-order sync
                # ring: a parallel ring races the weight stream for the
                # shared ~250GB/s DMA bandwidth and starves it (measured
                # 5.5us PE gap + a HAM re-throttle mid-chunk-0).
                nc.sync.dma_start(xt_sb, xt_r[:, nt, :, toff:toff + tch])
                xt_tiles.append(xt_sb)

            issue_xt(1)

            prev_ht = None  # previous chunk's scan output (carries the state)
            prev_tch = TCH

            def gates_scan_store(m, t0, tch, pk, pa, ht_sb, split=1):
                # split>1 processes the tile in column slices so the very
                # last tile's gate/scan/DMA chain after the final matmul is
                # short. ACT order s -> z: s feeds the longest chain
                # (g -> v -> scan). For the split (= final) tile the two
                # slice DMAs go out on different DGE rings so their ~600ns
                # descriptor generations run in parallel.
                w = tch // split
                for si in range(split):
                    lo, hi = si * w, (si + 1) * w
                    s_sb = gate_pool.tile([P, TCH], fp16, tag="s", name="s")[:, :w]
                    nc.scalar.activation(s_sb, pa[:, lo:hi], SIG, bias=bh_sb[:, m:m + 1])
                    z_sb = gate_pool.tile([P, TCH], fp16, tag="z", name="z")[:, :w]
                    nc.scalar.activation(z_sb, pk[:, lo:hi], SIG, bias=bz_sb[:, m:m + 1])
                    # g = max(a + bh + 0.5, s) straight from PSUM
                    g_sb = gate_pool.tile([P, TCH], fp16, tag="g", name="g")[:, :w]
                    nc.vector.scalar_tensor_tensor(
                        g_sb, pa[:, lo:hi], bhp5_sb[:, m:m + 1], s_sb, op0=ADD, op1=MAX
                    )
                    c_sb = gate_pool.tile([P, TCH], fp16, tag="c", name="c")[:, :w]
                    nc.vector.tensor_scalar(c_sb, z_sb, -1.0, 1.0, op0=MUL, op1=ADD)
                    v_sb = gate_pool.tile([P, TCH], fp16, tag="v", name="v")[:, :w]
                    nc.vector.tensor_mul(v_sb, z_sb, g_sb)

                    init = (
                        gh0_sb[:, m:m + 1]
                        if prev_ht is None
                        else prev_ht[:, m, prev_tch - 1:prev_tch]
                    ) if si == 0 else ht_sb[:, m, lo - 1:lo]
                    nc.vector.tensor_tensor_scan(
                        ht_sb[:, m, lo:hi], c_sb, v_sb, init, op0=MUL, op1=ADD
                    )
                    # The last m-tile's outputs ride the ACT ring: it keeps
                    # that ring warm all run (a ~200us-idle ring pays ~1us
                    # of restart latency on the final, fully-exposed DMA)
                    # and the final two slice DMAs' ~600ns descriptor
                    # generations run on different engines in parallel.
                    if split > 1 and si == split - 1:
                        # very last output: halves ride BOTH rings so the
                        # two DGE gens, packet streams and completion-sem
                        # posts run in parallel
                        mid = lo + w // 2
                        nc.sync.dma_start(
                            out_ext[m * P:(m + 1) * P, t0 + lo:t0 + mid],
                            ht_sb[:, m, lo:mid],
                        )
                        nc.scalar.dma_start(
                            out_ext[m * P:(m + 1) * P, t0 + mid:t0 + hi],
                            ht_sb[:, m, mid:hi],
                        )
                    else:
                        eng = nc.scalar if (m == MO - 1 and split == 1) else nc.sync
                        eng.dma_start(
                            out_ext[m * P:(m + 1) * P, t0 + lo:t0 + hi],
                            ht_sb[:, m, lo:hi],
                        )

            for ci, (t0, tch) in enumerate(CHUNKS):
                if ci + 2 < len(CHUNKS):
                    issue_xt(ci + 2)
                xt_sb = xt_tiles[ci]
                ht_sb = ht_pool.tile([P, MO, TCH], fp16)

                if ci == 0:
                    # k-outer over groups of 4 output tiles: matmuls consume
                    # the weight k-slices in DMA arrival order, so the PE
                    # starts ~15us earlier and never stalls on weight loads
                    # (which would also re-throttle the HAM clock gate).
                    # ko=0 of the first group is further split into t-halves
                    # so the very first matmuls wait on only half of x
                    # slice 0 (the second half's DMA finishes under them).
                    GQ = min(4, MO)
                    for half in range(MO // GQ):
                        pks = [
                            psum_p.tile([P, TCH], fp32, tag="pk", name="pk")
                            for _ in range(GQ)
                        ]
                        pas = [
                            psum_p.tile([P, TCH], fp32, tag="pa", name="pa")
                            for _ in range(GQ)
                        ]
                        # ALL pa (wh) matmuls of the half first, then all
                        # pk (wz): the early window only needs one weight
                        # matrix streamed. pa first also because s/g/v
                        # feed the longest downstream chain.
                        for ps, wsb in ((pas, wh_sb), (pks, wz_sb)):
                            for ko in range(KO):
                                if ko == 0 and half == 0:
                                    # t-split: half0 with start=True clears
                                    # the whole bank's has_written bits;
                                    # half1 (start=False) then overwrites
                                    # its cleared columns, and ko>=1
                                    # accumulates everywhere.
                                    for th in range(2):
                                        sl = slice(th * TH, (th + 1) * TH)
                                        for q in range(GQ):
                                            m = half * GQ + q
                                            nc.tensor.matmul(
                                                ps[q][:, sl],
                                                wsb[:, 0, m * P:(m + 1) * P],
                                                xt_sb[:, 0, sl],
                                                start=(th == 0),
                                                stop=False,
                                            )
                                    continue
                                for q in range(GQ):
                                    m = half * GQ + q
                                    nc.tensor.matmul(
                                        ps[q],
                                        wsb[:, ko, m * P:(m + 1) * P],
                                        xt_sb[:, ko, :],
                                        start=(ko == 0),
                                        stop=(ko == KO - 1),
                                    )
                        for q in range(GQ):
                            m = half * GQ + q
                            gates_scan_store(m, t0, tch, pks[q], pas[q], ht_sb)
                else:
                    for m in range(MO):
                        pk = psum_p.tile([P, TCH], fp32, tag="pk", name="pk")[:, :tch]
                        pa = psum_p.tile([P, TCH], fp32, tag="pa", name="pa")[:, :tch]
                        for ko in range(KO):
                            nc.tensor.matmul(
                                pa,
                                wh_sb[:, ko, m * P:(m + 1) * P],
                                xt_sb[:, ko, :tch],
                                start=(ko == 0),
                                stop=(ko == KO - 1),
                            )
                        for ko in range(KO):
                            nc.tensor.matmul(
                                pk,
                                wz_sb[:, ko, m * P:(m + 1) * P],
                                xt_sb[:, ko, :tch],
                                start=(ko == 0),
                                stop=(ko == KO - 1),
                            )
                        last = ci == len(CHUNKS) - 1 and m == MO - 1
                        gates_scan_store(m, t0, tch, pk, pa, ht_sb,
                                         split=2 if last else 1)

                prev_ht = ht_sb
                prev_tch = tch

    nc.finalize()
    return nc


def _get_program():
    if "v16" not in _PROGRAM_CACHE:
        _PROGRAM_CACHE["v16"] = _build_program()
    return _PROGRAM_CACHE["v16"]


def _prep_xt(xb):
    # [T, D] fp32 -> fp16 [ki, nt, ko, tch] with D = ko*128+ki, T = nt*512+tch
    x16 = np.asarray(xb, dtype=np.float16)
    xt = x16.reshape(NTCH, TCH, KO, P).transpose(3, 0, 2, 1)
    return np.ascontiguousarray(xt).reshape(P, NTCH * KO * TCH)


def run(x, h_0, Wz, bz, Wh, bh, trace=False):
    from concourse.bass_utils import run_bass_kernel_spmd

    nc = _get_program()
    wz16 = np.ascontiguousarray(np.asarray(Wz, dtype=np.float16))
    wh16 = np.ascontiguousarray(np.asarray(Wh, dtype=np.float16))
    bz32 = np.asarray(bz, dtype=np.float32)
    bh32 = np.asarray(bh, dtype=np.float32)
    h0_32 = np.asarray(h_0, dtype=np.float32).reshape(B, H)

    def dev_cols(v):  # [H] -> [P, MO] device layout (partition = mi)
        return v.reshape(MO, P).T

    in_maps = [
        {
            "xt": _prep_xt(x[b]),
            "Wz": wz16,
            "Wh": wh16,
            "cst": np.ascontiguousarray(np.concatenate(
                [dev_cols(bz32), dev_cols(bh32), dev_cols(h0_32[b])], axis=1
            )),
        }
        for b in range(B)
    ]
    res = run_bass_kernel_spmd(nc, in_maps, list(range(B)), trace=trace)
    out = np.empty((B, T + 1, H), dtype=np.float32)
    # h[0] = g(h_0) computed on host in fp32
    out[:, 0, :] = np.where(
        h0_32 >= 0.0, h0_32 + 0.5, 1.0 / (1.0 + np.exp(-h0_32))
    )
    for b in range(B):
        out[b, 1:, :] = res.results[b]["out"].T.astype(np.float32)
    return out, res


def kernel(x, h_0, Wz, bz, Wh, bh):
    out, _ = run(x, h_0, Wz, bz, Wh, bh)
    return out



# revision 39
# speedup vs baseline: 1.0139x; 1.0007x over previous
"""MinGRU forward on 8 Trainium2 NeuronCores.

Reference computation (per batch b):
    k       = x @ Wz + bz                 # [T, H]
    z       = sigmoid(k)
    c       = 1 - z
    htilde  = g(x @ Wh + bh)              # g(a) = a+0.5 if a>=0 else sigmoid(a)
                                          #      = max(a+0.5, sigmoid(a))
    h[0]    = g(h_0)
    h[t]    = c[t-1]*h[t-1] + z[t-1]*htilde[t-1]   (t = 1..T)
    out     = h                           # [T+1, H]

The log-space cumlogsumexp in the reference is exactly this linear
recurrence (all quantities positive, coefficients in (0,1), so the
linear form is numerically stable).

Sharding: data-parallel over batch, one batch per core, weights
replicated.

The kernel is Tensor-engine bound: 1024 fp16 matmuls/core = 218.5us at
2.4GHz. fp8 DoubleRow was measured on hardware at ~1 cycle/output-row
(2x FLOPs via 256-deep contraction, not the cost model's 4x), so
error-compensated hi/lo fp8 (3 logical matmuls, verified numerically
at 0.008 max rel err) is 1.5x SLOWER than fp16 — fp16 is optimal.
The optimization is therefore all PE-occupancy at the edges
(257.7us -> 245.1us measured):
  - x is transposed AND cast to fp16 on the host, so the device issues
    only plain contiguous DMAs. The baseline's device-side DMA-transpose
    serialized the weight loads behind it, costing ~8us of PE idle at
    kernel start.
  - DMA priority: weights stream ALONE on the in-order sync ring,
    ordered exactly as chunk 0's k-outer schedule consumes them (all
    low m-halves first, wh0's low half split so the first matmul waits
    on 32KB). ALL x traffic (chunk 0 slice-by-slice, then whole
    chunks) rides the ACT ring — interleaving chunk-0 x into the
    weight ring starved the weight stream mid-chunk-0 (~1us of PE
    gaps at the lo->hi half switch); x slice 0 is split in halves and
    ko=0's matmuls t-split so the PE starts on the first 64KB.
    Small constants are host-packed into one [128, 24] block — the
    natural per-vector rearranges emit 1024 4-byte scatter descriptors
    that starve the critical first weight slice.
  - Gates run fp16 end to end (z, s, c, g, v, h): DVE gets 2x
    throughput on 16-bit SBUF operands, the output DMA halves, and ACT
    drops from 3 sigmoids to 2 (c = 1-z moves to a cheap DVE
    tensor_scalar). GpSimd compute is not used at all (its software
    multiply is ~4x slower than DVE fp16).
  - The scan keeps fp32 state internally (hardware guarantee) and only
    stores h as fp16; rel err stays ~4.6e-3 (limit 2e-2).
  - The last 512 timesteps run as two 256 chunks and the final tile's
    gates in two 128 slices, so the post-matmul tail chain is short.
  - HAM pre-warm: 18 ZERO-data matmuls bridge the preamble-to-first-
    data window so the clock gate flips to 2.4GHz right at the real
    stream's start. Zeroed operands = no array switching, so this does
    NOT trip the power envelope the way a warmup on real data did
    (+38us, 13/16-duty); measured clean 4/8 -> 8/8 transitions.
  - Chunk 0 is marginally DMA-bound (~5MB of weights+x must stream
    during its ~27.6us of matmuls at ~200GB/s early ring speed), so
    chunk 0 runs ALL wh matmuls of a half before any wz matmuls and
    the weight stream is ordered to match: the early window then only
    has to keep up with one weight matrix while the pipeline fills.
The device writes timesteps 1..T transposed ([H, T] fp16); the host
prepends g(h_0), transposes and upcasts during the unshard.
"""

import numpy as np

B, T, D, H = 8, 4096, 1024, 1024
P = 128
TCH = 512                 # time-chunk (one PSUM bank of fp32 per matmul)
KO = D // P               # contraction tiles
MO = H // P               # output-channel tiles
# 7 full chunks + 2 half chunks at the end to shorten the tail
CHUNKS = [(i * TCH, TCH) for i in range(7)] + [(3584, 256), (3840, 256)]
NTCH = T // TCH           # host x layout is uniform 512-chunk-major

_PROGRAM_CACHE = {}


def _build_program():
    import concourse.bacc as bacc
    import concourse.mybir as mybir
    import concourse.tile as tile

    fp32 = mybir.dt.float32
    fp16 = mybir.dt.float16
    SIG = mybir.ActivationFunctionType.Sigmoid
    MUL = mybir.AluOpType.mult
    ADD = mybir.AluOpType.add
    MAX = mybir.AluOpType.max

    nc = bacc.Bacc("TRN2", target_bir_lowering=False)

    # x pre-transposed on host: [ki, nt, ko, t] with D-index = ko*128+ki,
    # T-index = nt*512+t  (chunk-major so each chunk DMA reads 8KB runs)
    xt_ext = nc.declare_dram_parameter("xt", [P, NTCH * KO * TCH], fp16, isOutput=False)
    wz_ext = nc.declare_dram_parameter("Wz", [D, H], fp16, isOutput=False)
    wh_ext = nc.declare_dram_parameter("Wh", [D, H], fp16, isOutput=False)
    # host-packed [bz_t | bh_t | h0_t] in device layout (partition = channel
    # within tile, free = tile): a single small contiguous DMA. The natural
    # per-tensor rearranges generate 1024 4-byte scatter descriptors each,
    # which hogged the DMA engines right when the first weight slice's bulk
    # data needed them.
    cst_ext = nc.declare_dram_parameter("cst", [P, 3 * MO], fp32, isOutput=False)
    # transposed fp16 output, timesteps 1..T; the host prepends g(h_0) and
    # untransposes/upcasts during the gather
    out_ext = nc.declare_dram_parameter("out", [H, T], fp16, isOutput=True)

    xt_r = xt_ext.rearrange("p (nt ko t) -> p nt ko t", nt=NTCH, ko=KO)

    with tile.TileContext(nc) as tc:
        with (
            tc.tile_pool(name="const", bufs=1) as const_pool,
            tc.tile_pool(name="w", bufs=1) as w_pool,
            tc.tile_pool(name="xt", bufs=3) as xt_pool,
            tc.tile_pool(name="ht", bufs=2) as ht_pool,
            tc.tile_pool(name="gate", bufs=6) as gate_pool,
            tc.tile_pool(name="psp", bufs=4, space="PSUM") as psum_p,
        ):
            # HAM pre-warm: ~13 zero-data matmuls run back-to-back from
            # right after the preamble until the first real matmul's data
            # lands (~3.5us in). The PE activity window then flips the
            # clock gate to 8/8 (2.4GHz) ~4-6us into the window instead
            # of ~10us, shaving 1-2.5us of half-rate matmuls. Zeroed
            # operands keep array switching (= power draw) near nil, so
            # this does not trip the power-envelope throttle the way a
            # warmup on real data during the DMA burst did (+38us).
            warm_sb = const_pool.tile([P, 384], fp16)
            nc.gpsimd.memset(warm_sb, 0.0)
            warm_ps = psum_p.tile([P, TCH], fp32, tag="pk", name="pk")
            for _ in range(18):
                nc.tensor.matmul(
                    warm_ps[:, 0:256], warm_sb[:, 0:P], warm_sb[:, P:P + 256],
                    start=True, stop=True,
                )

            # Chunk 0's x rides the ACT ring, in two halves so the very
            # first matmuls (split into t-halves for ko=0) wait on only
            # 64KB. The sync ring carries ONLY weights in its early
            # window: interleaving chunk-0 x there starved the weight
            # stream mid-chunk-0 (~1us of PE gaps at the lo->hi switch).
            xt_first = xt_pool.tile([P, KO, TCH], fp16, tag="xt512", name="xt512")
            TH = TCH // 2
            nc.scalar.dma_start(xt_first[:, 0, 0:TH], xt_r[:, 0, 0, 0:TH])
            nc.scalar.dma_start(xt_first[:, 0, TH:], xt_r[:, 0, 0, TH:])

            # Weights resident: [ki, ko, h] so lhsT tiles are natural slices.
            # Loaded per k-slice (contiguous 256KB each) on the sync ring, in
            # the order chunk 0's k-outer matmul schedule consumes them
            # (pa/wh first). The sync ring carries nothing else early, so
            # the ~130GB/s the stream needs is comfortably under the ring's
            # ~200GB/s and the PE never waits on a weight slice.
            wz_sb = w_pool.tile([P, KO, H], fp16)
            wh_sb = w_pool.tile([P, KO, H], fp16)
            wz_r = wz_ext.rearrange("(ko ki) h -> ki ko h", ki=P)
            wh_r = wh_ext.rearrange("(ko ki) h -> ki ko h", ki=P)
            # Stream order matches chunk 0's k-outer consumption exactly:
            # the first half (m-tiles 0-3) of every k-slice first — wh0's
            # low half further split so the very first matmuls wait on
            # 32KB/96KB — then all high halves. Half 0 of chunk 0 then
            # needs only 2MB of weights in its window instead of 4MB.
            # Chunk 0 runs ALL wh (pa) matmuls of a half before any wz
            # (pk) matmuls, so the early stream only has to keep up with
            # ONE weight matrix (~145GB/s) while the pipeline fills —
            # with a warm (pre-warmed) PE the interleaved order outran
            # the stream and left ~2us of gaps. Stream order matches.
            HH = H // 2
            nc.sync.dma_start(wh_sb[:, 0, 0:P], wh_r[:, 0, 0:P])
            nc.sync.dma_start(wh_sb[:, 0, P:HH], wh_r[:, 0, P:HH])
            for ko in range(1, KO):
                nc.sync.dma_start(wh_sb[:, ko, 0:HH], wh_r[:, ko, 0:HH])
            for ko in range(KO):
                nc.sync.dma_start(wz_sb[:, ko, 0:HH], wz_r[:, ko, 0:HH])
            for ko in range(KO):
                nc.sync.dma_start(wh_sb[:, ko, HH:], wh_r[:, ko, HH:])
            for ko in range(KO):
                nc.sync.dma_start(wz_sb[:, ko, HH:], wz_r[:, ko, HH:])

            # Chunk-0 x slices ko=1..7 follow on the ACT ring (ko=1 is
            # needed ~1us after the first matmul); the small constants
            # DMA rides behind them (first needed only at the gates,
            # ~10us later).
            for ko in range(1, KO):
                nc.scalar.dma_start(xt_first[:, ko], xt_r[:, 0, ko])
            cst_sb = const_pool.tile([P, 3 * MO], fp32)
            nc.scalar.dma_start(cst_sb, cst_ext[:, :])
            bz_sb = cst_sb[:, 0:MO]
            bh_sb = cst_sb[:, MO:2 * MO]
            h0_sb = cst_sb[:, 2 * MO:3 * MO]
            bhp5_sb = const_pool.tile([P, MO], fp32)
            nc.vector.tensor_scalar_add(bhp5_sb, bh_sb, 0.5)

            # g(h_0) for the chunk-0 scan init (out column 0 is host-side)
            s0_sb = const_pool.tile([P, MO], fp32)
            nc.scalar.activation(s0_sb, h0_sb, SIG)
            gh0_sb = const_pool.tile([P, MO], fp32)
            nc.vector.scalar_tensor_tensor(gh0_sb, h0_sb, 0.5, s0_sb, op0=ADD, op1=MAX)

            # x chunks prefetched two ahead on the GpSimd ring. Tiles are
            # tagged by width so the 256-wide tail chunks rotate separately.
            xt_tiles = [xt_first]

            def issue_xt(ci):
                t0, tch = CHUNKS[ci]
                xt_sb = xt_pool.tile(
                    [P, KO, tch], fp16, tag=f"xt{tch}", name=f"xt{tch}"
                )
                nt, toff = divmod(t0, TCH)
                # chunks 1+ queue BEHIND the weights on the in-order sync
                # ring: a parallel ring races the weight stream for the
                # shared ~250GB/s DMA bandwidth and starves it (measured
                # 5.5us PE gap + a HAM re-throttle mid-chunk-0).
                nc.sync.dma_start(xt_sb, xt_r[:, nt, :, toff:toff + tch])
                xt_tiles.append(xt_sb)

            issue_xt(1)

            prev_ht = None  # previous chunk's scan output (carries the state)
            prev_tch = TCH

            def gates_scan_store(m, t0, tch, pk, pa, ht_sb, split=1):
                # split>1 processes the tile in column slices so the very
                # last tile's gate/scan/DMA chain after the final matmul is
                # short. ACT order s -> z: s feeds the longest chain
                # (g -> v -> scan). For the split (= final) tile the two
                # slice DMAs go out on different DGE rings so their ~600ns
                # descriptor generations run in parallel.
                w = tch // split
                for si in range(split):
                    lo, hi = si * w, (si + 1) * w
                    s_sb = gate_pool.tile([P, TCH], fp16, tag="s", name="s")[:, :w]
                    nc.scalar.activation(s_sb, pa[:, lo:hi], SIG, bias=bh_sb[:, m:m + 1])
                    z_sb = gate_pool.tile([P, TCH], fp16, tag="z", name="z")[:, :w]
                    nc.scalar.activation(z_sb, pk[:, lo:hi], SIG, bias=bz_sb[:, m:m + 1])
                    # g = max(a + bh + 0.5, s) straight from PSUM
                    g_sb = gate_pool.tile([P, TCH], fp16, tag="g", name="g")[:, :w]
                    nc.vector.scalar_tensor_tensor(
                        g_sb, pa[:, lo:hi], bhp5_sb[:, m:m + 1], s_sb, op0=ADD, op1=MAX
                    )
                    c_sb = gate_pool.tile([P, TCH], fp16, tag="c", name="c")[:, :w]
                    nc.vector.tensor_scalar(c_sb, z_sb, -1.0, 1.0, op0=MUL, op1=ADD)
                    v_sb = gate_pool.tile([P, TCH], fp16, tag="v", name="v")[:, :w]
                    nc.vector.tensor_mul(v_sb, z_sb, g_sb)

                    init = (
                        gh0_sb[:, m:m + 1]
                        if prev_ht is None
                        else prev_ht[:, m, prev_tch - 1:prev_tch]
                    ) if si == 0 else ht_sb[:, m, lo - 1:lo]
                    nc.vector.tensor_tensor_scan(
                        ht_sb[:, m, lo:hi], c_sb, v_sb, init, op0=MUL, op1=ADD
                    )
                    # The last m-tile's outputs ride the ACT ring: it keeps
                    # that ring warm all run (a ~200us-idle ring pays ~1us
                    # of restart latency on the final, fully-exposed DMA)
                    # and the final two slice DMAs' ~600ns descriptor
                    # generations run on different engines in parallel.
                    if split > 1:
                        # final tile: alternate rings so the two DGEs overlap
                        eng = nc.scalar if si == split - 1 else nc.sync
                    else:
                        eng = nc.scalar if m == MO - 1 else nc.sync
                    eng.dma_start(
                        out_ext[m * P:(m + 1) * P, t0 + lo:t0 + hi],
                        ht_sb[:, m, lo:hi],
                    )

            for ci, (t0, tch) in enumerate(CHUNKS):
                if ci + 2 < len(CHUNKS):
                    issue_xt(ci + 2)
                xt_sb = xt_tiles[ci]
                ht_sb = ht_pool.tile([P, MO, TCH], fp16)

                if ci == 0:
                    # k-outer over groups of 4 output tiles: matmuls consume
                    # the weight k-slices in DMA arrival order, so the PE
                    # starts ~15us earlier and never stalls on weight loads
                    # (which would also re-throttle the HAM clock gate).
                    # ko=0 of the first group is further split into t-halves
                    # so the very first matmuls wait on only half of x
                    # slice 0 (the second half's DMA finishes under them).
                    GQ = min(4, MO)
                    for half in range(MO // GQ):
                        pks = [
                            psum_p.tile([P, TCH], fp32, tag="pk", name="pk")
                            for _ in range(GQ)
                        ]
                        pas = [
                            psum_p.tile([P, TCH], fp32, tag="pa", name="pa")
                            for _ in range(GQ)
                        ]
                        # ALL pa (wh) matmuls of the half first, then all
                        # pk (wz): the early window only needs one weight
                        # matrix streamed. pa first also because s/g/v
                        # feed the longest downstream chain.
                        for ps, wsb in ((pas, wh_sb), (pks, wz_sb)):
                            for ko in range(KO):
                                if ko == 0 and half == 0:
                                    # t-split: half0 with start=True clears
                                    # the whole bank's has_written bits;
                                    # half1 (start=False) then overwrites
                                    # its cleared columns, and ko>=1
                                    # accumulates everywhere.
                                    for th in range(2):
                                        sl = slice(th * TH, (th + 1) * TH)
                                        for q in range(GQ):
                                            m = half * GQ + q
                                            nc.tensor.matmul(
                                                ps[q][:, sl],
                                                wsb[:, 0, m * P:(m + 1) * P],
                                                xt_sb[:, 0, sl],
                                                start=(th == 0),
                                                stop=False,
                                            )
                                    continue
                                for q in range(GQ):
                                    m = half * GQ + q
                                    nc.tensor.matmul(
                                        ps[q],
                                        wsb[:, ko, m * P:(m + 1) * P],
                                        xt_sb[:, ko, :],
                                        start=(ko == 0),
                                        stop=(ko == KO - 1),
                                    )
                        for q in range(GQ):
                            m = half * GQ + q
                            gates_scan_store(m, t0, tch, pks[q], pas[q], ht_sb)
                else:
                    for m in range(MO):
                        pk = psum_p.tile([P, TCH], fp32, tag="pk", name="pk")[:, :tch]
                        pa = psum_p.tile([P, TCH], fp32, tag="pa", name="pa")[:, :tch]
                        for ko in range(KO):
                            nc.tensor.matmul(
                                pa,
                                wh_sb[:, ko, m * P:(m + 1) * P],
                                xt_sb[:, ko, :tch],
                                start=(ko == 0),
                                stop=(ko == KO - 1),
                            )
                        for ko in range(KO):
                            nc.tensor.matmul(
                                pk,
                                wz_sb[:, ko, m * P:(m + 1) * P],
                                xt_sb[:, ko, :tch],
                                start=(ko == 0),
                                stop=(ko == KO - 1),
                            )
                        last = ci == len(CHUNKS) - 1 and m == MO - 1
                        gates_scan_store(m, t0, tch, pk, pa, ht_sb,
                                         split=2 if last else 1)

                prev_ht = ht_sb
                prev_tch = tch

    nc.finalize()
    return nc


def _get_program():
    if "v9" not in _PROGRAM_CACHE:
        _PROGRAM_CACHE["v9"] = _build_program()
    return _PROGRAM_CACHE["v9"]


def _prep_xt(xb):
    # [T, D] fp32 -> fp16 [ki, nt, ko, tch] with D = ko*128+ki, T = nt*512+tch
    x16 = np.asarray(xb, dtype=np.float16)
    xt = x16.reshape(NTCH, TCH, KO, P).transpose(3, 0, 2, 1)
    return np.ascontiguousarray(xt).reshape(P, NTCH * KO * TCH)


def run(x, h_0, Wz, bz, Wh, bh, trace=False):
    from concourse.bass_utils import run_bass_kernel_spmd

    nc = _get_program()
    wz16 = np.ascontiguousarray(np.asarray(Wz, dtype=np.float16))
    wh16 = np.ascontiguousarray(np.asarray(Wh, dtype=np.float16))
    bz32 = np.asarray(bz, dtype=np.float32)
    bh32 = np.asarray(bh, dtype=np.float32)
    h0_32 = np.asarray(h_0, dtype=np.float32).reshape(B, H)

    def dev_cols(v):  # [H] -> [P, MO] device layout (partition = mi)
        return v.reshape(MO, P).T

    in_maps = [
        {
            "xt": _prep_xt(x[b]),
            "Wz": wz16,
            "Wh": wh16,
            "cst": np.ascontiguousarray(np.concatenate(
                [dev_cols(bz32), dev_cols(bh32), dev_cols(h0_32[b])], axis=1
            )),
        }
        for b in range(B)
    ]
    res = run_bass_kernel_spmd(nc, in_maps, list(range(B)), trace=trace)
    out = np.empty((B, T + 1, H), dtype=np.float32)
    # h[0] = g(h_0) computed on host in fp32
    out[:, 0, :] = np.where(
        h0_32 >= 0.0, h0_32 + 0.5, 1.0 / (1.0 + np.exp(-h0_32))
    )
    for b in range(B):
        out[b, 1:, :] = res.results[b]["out"].T.astype(np.float32)
    return out, res


def kernel(x, h_0, Wz, bz, Wh, bh):
    out, _ = run(x, h_0, Wz, bz, Wh, bh)
    return out

